# revision 1
# baseline (speedup 1.0000x reference)
"""Trainium2 Bass kernel: LoRA-LiME embedding with MoE routing.

Computes, for input_ids [B,T] over an embedding table [V,H]:
    E   = emb[ids]                                  # gather
    t   = E @ A.T ; delta = t @ B.T * scaling       # LoRA (rank 16)
    routing on first-8 feature slices with GLOBAL abs-max scales
    soft top-2 mask, renormalized expert weights
    p   = (1-g) * w @ LiMEs + g * LiME_shared       # g = sigmoid(gamma)
    out = E + delta * p

Sharding: data-parallel over the 8192 tokens (1024/core on 8 cores),
embedding table replicated. The global abs-max pair is an in-NEFF
AllReduce(max) over the 8 cores.
"""

import numpy as np

from concourse import bacc, bass, mybir, tile
from concourse import bass_utils
from concourse.masks import make_identity

F32 = mybir.dt.float32
I32 = mybir.dt.int32
ALU = mybir.AluOpType
ACTF = mybir.ActivationFunctionType
P = 128


class Cfg:
    def __init__(self, vocab=50257, h=2048, tpc=1024, n_cores=8, n_experts=8,
                 rank=16, gamma_routing=0.5, soft_topk_temp=0.5, eps=1e-6,
                 temperature=1.0):
        assert h % P == 0 and tpc % P == 0
        self.vocab, self.h, self.tpc, self.n_cores = vocab, h, tpc, n_cores
        self.e, self.r = n_experts, rank
        self.gamma_routing = gamma_routing
        self.soft_topk_temp = soft_topk_temp
        self.eps, self.temperature = eps, temperature
        self.nt = tpc // P                  # token tiles per core
        self.nch = h // P                   # 128-wide H chunks (transpose/stage1)
        self.ndc = (h + 511) // 512         # 512-wide N chunks (stage2/pmix)


FULL = Cfg()


def build_program(cfg: Cfg):
    nc = bacc.Bacc("TRN2", target_bir_lowering=False, debug=False,
                   num_devices=cfg.n_cores)

    ids = nc.dram_tensor("ids", [cfg.tpc, 1], I32, kind="ExternalInput").ap()
    emb = nc.dram_tensor("emb", [cfg.vocab, cfg.h], F32, kind="ExternalInput").ap()
    a_t = nc.dram_tensor("a_t", [cfg.h, cfg.r], F32, kind="ExternalInput").ap()
    b_t = nc.dram_tensor("b_t", [cfg.r, cfg.h], F32, kind="ExternalInput").ap()
    limes = nc.dram_tensor("limes", [16, cfg.h], F32, kind="ExternalInput").ap()
    gamma = nc.dram_tensor("gamma", [1, 1], F32, kind="ExternalInput").ap()
    out = nc.dram_tensor("out", [cfg.tpc, cfg.h], F32, kind="ExternalOutput").ap()

    with tile.TileContext(nc) as tc:
        _body(nc, tc, cfg, ids, emb, a_t, b_t, limes, gamma, out)

    nc.compile()
    return nc


def _body(nc, tc, cfg, ids, emb, a_t, b_t, limes, gamma, out):
    E, R, H, NT, NCH, NDC = cfg.e, cfg.r, cfg.h, cfg.nt, cfg.nch, cfg.ndc
    inv_temp = 1.0 / cfg.temperature
    g_r = cfg.gamma_routing
    topk_slope = 1.0 / cfg.soft_topk_temp

    with (
        tc.tile_pool(name="const", bufs=1) as constp,
        tc.tile_pool(name="eout", bufs=NT) as eoutp,
        tc.tile_pool(name="delta", bufs=NT) as deltap,
        tc.tile_pool(name="work", bufs=2) as workp,
        tc.tile_pool(name="small", bufs=4) as smallp,
        tc.tile_pool(name="ps_big", bufs=3, space="PSUM") as ps_big,
        tc.tile_pool(name="ps_acc", bufs=2, space="PSUM") as ps_acc,
        tc.tile_pool(name="ps_sml", bufs=2, space="PSUM") as ps_sml,
        tc.tile_pool(name="dram", bufs=1, space="DRAM") as dramp,
    ):
        # ---- constants / params --------------------------------------
        ident = constp.tile([P, P], F32)
        make_identity(nc, ident[:])
        ones1 = constp.tile([1, P], F32)
        nc.vector.memset(ones1[:], 1.0)

        # A^T chunks: aT_sb[:, c*R:(c+1)*R] = A^T[c*128:(c+1)*128, :]
        aT_sb = constp.tile([P, NCH * R], F32)
        for c in range(NCH):
            nc.sync.dma_start(out=aT_sb[:, c * R:(c + 1) * R],
                              in_=a_t[c * P:(c + 1) * P, :])
        bT_sb = constp.tile([16, H], F32)
        nc.sync.dma_start(out=bT_sb[:R, :], in_=b_t[:, :])
        limes_sb = constp.tile([16, H], F32)
        nc.sync.dma_start(out=limes_sb[:], in_=limes[:, :])
        gm1 = constp.tile([1, 1], F32)
        nc.sync.dma_start(out=gm1[:], in_=gamma[:, :])

        # g = sigmoid(gamma) broadcast to all 128 partitions via K=1 matmul
        ps_g = ps_sml.tile([P, 2], F32, tag="ps_sml")
        nc.tensor.matmul(out=ps_g[:, 0:1], lhsT=ones1[:], rhs=gm1[:],
                         start=True, stop=True)
        gb = smallp.tile([P, 1], F32, tag="gtiles")
        nc.scalar.activation(out=gb[:], in_=ps_g[:, 0:1], func=ACTF.Sigmoid)
        omg = smallp.tile([P, 1], F32, tag="gtiles")
        nc.vector.tensor_scalar(out=omg[:], in0=gb[:], scalar1=-1.0, scalar2=1.0,
                                op0=ALU.mult, op1=ALU.add)

        # ---- resident accumulators -----------------------------------
        tT_all = constp.tile([16, NT * P], F32)      # t^T, all tiles
        esl_all = constp.tile([P, NT * E], F32)      # E[:, :8] per tile
        dsl_ps = ps_sml.tile([P, NT * E], F32, tag="ps_dsl", bufs=1)
        eouts, deltas = [], []

        # ================= phase A: gather + stage1 ===================
        for i in range(NT):
            ids_t = smallp.tile([P, 1], I32, tag="ids", name=f"ids{i}")
            nc.sync.dma_start(out=ids_t[:], in_=ids[i * P:(i + 1) * P, :])
            eo = eoutp.tile([P, H], F32, tag="eout", name=f"eout{i}")
            eouts.append(eo)
            nc.gpsimd.indirect_dma_start(
                out=eo[:], out_offset=None, in_=emb,
                in_offset=bass.IndirectOffsetOnAxis(ap=ids_t[:, :1], axis=0))

            nc.vector.tensor_copy(out=esl_all[:, i * E:(i + 1) * E],
                                  in_=eo[:, 0:E])

            # transpose E chunks (4 per PSUM bank), stage-1 accumulate
            eoT = workp.tile([P, H], F32, tag="eoT", name=f"eoT{i}")
            for c4 in range((NCH + 3) // 4):
                nch_here = min(4, NCH - c4 * 4)
                ps_tr = ps_big.tile([P, nch_here * P], F32, tag="ps_big",
                                    name=f"ps_tr{i}_{c4}")
                for j in range(nch_here):
                    ch = c4 * 4 + j
                    nc.tensor.transpose(out=ps_tr[:, j * P:(j + 1) * P],
                                        in_=eo[:, ch * P:(ch + 1) * P],
                                        identity=ident[:])
                nc.scalar.copy(out=eoT[:, c4 * 4 * P:c4 * 4 * P + nch_here * P],
                               in_=ps_tr[:])

            tps = ps_acc.tile([16, P], F32, tag="tps", name=f"tps{i}")
            for ch in range(NCH):
                nc.tensor.matmul(out=tps[:R, :],
                                 lhsT=aT_sb[:, ch * R:(ch + 1) * R],
                                 rhs=eoT[:, ch * P:(ch + 1) * P],
                                 start=(ch == 0), stop=(ch == NCH - 1))
            nc.vector.tensor_copy(out=tT_all[:, i * P:(i + 1) * P], in_=tps[:])

            # d_sl for this tile (needs only b_t cols 0:E)
            nc.tensor.matmul(out=dsl_ps[:, i * E:(i + 1) * E],
                             lhsT=tT_all[:R, i * P:(i + 1) * P],
                             rhs=bT_sb[:R, 0:E], start=True, stop=True)

        # ================= phase B: global abs-max ====================
        dsl_sb = constp.tile([P, NT * E], F32)
        nc.vector.tensor_copy(out=dsl_sb[:], in_=dsl_ps[:])

        loc2 = smallp.tile([P, 2], F32, tag="loc")
        nc.vector.tensor_reduce(out=loc2[:, 0:1], in_=esl_all[:],
                                axis=mybir.AxisListType.X, op=ALU.max,
                                apply_absolute_value=True)
        nc.vector.tensor_reduce(out=loc2[:, 1:2], in_=dsl_sb[:],
                                axis=mybir.AxisListType.X, op=ALU.max,
                                apply_absolute_value=True)
        ps_l = ps_sml.tile([2, P], F32, tag="ps_sml")
        nc.tensor.transpose(out=ps_l[:], in_=loc2[:], identity=ident[:])
        l2T = smallp.tile([2, P], F32, tag="loc")
        nc.vector.tensor_copy(out=l2T[:], in_=ps_l[:])
        lmax = smallp.tile([2, 1], F32, tag="loc")
        nc.vector.tensor_reduce(out=lmax[:], in_=l2T[:],
                                axis=mybir.AxisListType.X, op=ALU.max)

        cc_in = dramp.tile([2, 1], F32)
        cc_out = dramp.tile(
            [2, 1], F32,
            addr_space="Shared" if cfg.n_cores > 4 else "Local")
        nc.sync.dma_start(out=cc_in[:], in_=lmax[:])
        nc.gpsimd.collective_compute(
            "AllReduce", ALU.max,
            replica_groups=[list(range(cfg.n_cores))],
            ins=[cc_in[:].opt()], outs=[cc_out[:].opt()])
        sc01 = smallp.tile([1, 2], F32, tag="loc")
        nc.sync.dma_start(out=sc01[:], in_=cc_out[:].rearrange("a b -> b a"))

        # broadcast scales to 128 partitions; sc2 = 0.5/max(scale, eps)
        ps_b = ps_sml.tile([P, 2], F32, tag="ps_sml")
        nc.tensor.matmul(out=ps_b[:], lhsT=ones1[:], rhs=sc01[:],
                         start=True, stop=True)
        sc2 = smallp.tile([P, 2], F32, tag="loc")
        nc.vector.tensor_scalar_max(sc2[:], ps_b[:], cfg.eps)
        nc.vector.reciprocal(out=sc2[:], in_=sc2[:])
        nc.vector.tensor_scalar_mul(sc2[:], sc2[:], g_r * inv_temp)
        sc2e = smallp.tile([P, 2], F32, tag="loc")
        nc.vector.tensor_scalar_mul(sc2e[:], sc2[:], (1.0 - g_r) / g_r)

        # ============ phase B2: stage-2 delta (overlaps collective) ===
        for i in range(NT):
            dl = deltap.tile([P, H], F32, tag="delta", name=f"delta{i}")
            deltas.append(dl)
            for c in range(NDC):
                n0, n1 = c * 512, min((c + 1) * 512, H)
                ps_dl = ps_big.tile([P, n1 - n0], F32, tag="ps_big",
                                    name=f"ps_dl{i}_{c}")
                nc.tensor.matmul(out=ps_dl[:], lhsT=tT_all[:R, i * P:(i + 1) * P],
                                 rhs=bT_sb[:R, n0:n1], start=True, stop=True)
                nc.scalar.copy(out=dl[:, n0:n1], in_=ps_dl[:])

        # ================= phase C: routing + output ==================
        dsc = constp.tile([P, NT * E], F32)
        nc.vector.tensor_scalar(out=dsc[:], in0=dsl_sb[:], scalar1=sc2[:, 1:2],
                                scalar2=None, op0=ALU.mult)
        logits = constp.tile([P, NT * E], F32)
        nc.vector.scalar_tensor_tensor(out=logits[:], in0=esl_all[:],
                                       scalar=sc2e[:, 0:1], in1=dsc[:],
                                       op0=ALU.mult, op1=ALU.add)
        e_all = constp.tile([P, NT * E], F32)
        nc.scalar.activation(out=e_all[:], in_=logits[:], func=ACTF.Exp)
        s8 = smallp.tile([P, NT], F32, tag="r8")
        nc.vector.tensor_reduce(
            out=s8[:], in_=e_all[:].rearrange("p (t e) -> p t e", e=E),
            axis=mybir.AxisListType.X, op=ALU.add)
        rs2 = smallp.tile([P, NT], F32, tag="r8")
        nc.vector.reciprocal(out=rs2[:], in_=s8[:])
        nc.vector.tensor_scalar_mul(rs2[:], rs2[:], topk_slope)

        thr = smallp.tile([P, NT], F32, tag="r8")
        for i in range(NT):
            m8 = smallp.tile([P, 8], F32, tag="m8", bufs=2, name=f"m8_{i}")
            nc.vector.max(out=m8[:], in_=e_all[:, i * E:(i + 1) * E])
            nc.vector.tensor_copy(out=thr[:, i:i + 1], in_=m8[:, 1:2])
        bias_all = smallp.tile([P, NT], F32, tag="r8")
        nc.vector.tensor_mul(bias_all[:], thr[:], rs2[:])
        nc.vector.tensor_scalar_mul(bias_all[:], bias_all[:], -1.0)

        mask_all = constp.tile([P, NT * E], F32)
        for i in range(NT):
            nc.scalar.activation(out=mask_all[:, i * E:(i + 1) * E],
                                 in_=e_all[:, i * E:(i + 1) * E],
                                 func=ACTF.Sigmoid,
                                 bias=bias_all[:, i:i + 1],
                                 scale=rs2[:, i:i + 1])
        u_all = constp.tile([P, NT * E], F32)
        nc.vector.scalar_tensor_tensor(out=u_all[:], in0=e_all[:], scalar=0.0,
                                       in1=mask_all[:], op0=ALU.bypass,
                                       op1=ALU.mult)
        su8 = smallp.tile([P, NT], F32, tag="r8")
        nc.vector.tensor_reduce(
            out=su8[:], in_=u_all[:].rearrange("p (t e) -> p t e", e=E),
            axis=mybir.AxisListType.X, op=ALU.add)
        den8 = smallp.tile([P, NT], F32, tag="r8")
        nc.vector.scalar_tensor_tensor(out=den8[:], in0=s8[:], scalar=1e-9,
                                       in1=su8[:], op0=ALU.mult, op1=ALU.add)
        rdg8 = smallp.tile([P, NT], F32, tag="r8")
        nc.vector.reciprocal(out=rdg8[:], in_=den8[:])
        nc.vector.tensor_mul(rdg8[:], rdg8[:],
                             omg[:].to_broadcast([P, NT]))

        for i in range(NT):
            w16 = smallp.tile([P, 16], F32, tag="w16", name=f"w16_{i}")
            nc.vector.memset(w16[:], 0.0)
            nc.vector.tensor_scalar(out=w16[:, 0:E], in0=u_all[:, i * E:(i + 1) * E],
                                    scalar1=rdg8[:, i:i + 1], scalar2=None,
                                    op0=ALU.mult)
            nc.vector.tensor_copy(out=w16[:, E:E + 1], in_=gb[:])
            ps_w = ps_sml.tile([16, P], F32, tag="ps_sml", name=f"ps_w{i}")
            nc.tensor.transpose(out=ps_w[:], in_=w16[:], identity=ident[:])
            wT = smallp.tile([16, P], F32, tag="wT", name=f"wT{i}")
            nc.scalar.copy(out=wT[:], in_=ps_w[:])

            eo, dl = eouts[i], deltas[i]
            for c in range(NDC):
                n0, n1 = c * 512, min((c + 1) * 512, H)
                ps_pm = ps_big.tile([P, n1 - n0], F32, tag="ps_big",
                                    name=f"ps_pm{i}_{c}")
                nc.tensor.matmul(out=ps_pm[:], lhsT=wT[:], rhs=limes_sb[:, n0:n1],
                                 start=True, stop=True)
                tmp = workp.tile([P, n1 - n0], F32, tag="tmp", name=f"tmp{i}_{c}")
                nc.vector.tensor_mul(tmp[:], ps_pm[:], dl[:, n0:n1])
                nc.vector.tensor_add(eo[:, n0:n1], tmp[:], eo[:, n0:n1])
            nc.sync.dma_start(out=out[i * P:(i + 1) * P, :], in_=eo[:])


# ---------------------------------------------------------------------
# host entry point
# ---------------------------------------------------------------------
_CACHED = {}


def _get_program(cfg: Cfg):
    key = (cfg.vocab, cfg.h, cfg.tpc, cfg.n_cores)
    if key not in _CACHED:
        _CACHED[key] = build_program(cfg)
    return _CACHED[key]


def make_in_maps(cfg, input_ids, emb_weight, A, B_lora, LiMEs, LiME_shared, gamma,
                 scaling):
    ids_flat = np.asarray(input_ids).reshape(-1).astype(np.int32)
    emb_np = np.ascontiguousarray(np.asarray(emb_weight, dtype=np.float32))
    a_t = np.ascontiguousarray(np.asarray(A, dtype=np.float32).T)
    b_t = np.ascontiguousarray(np.asarray(B_lora, dtype=np.float32).T * scaling)
    limes16 = np.zeros((16, cfg.h), dtype=np.float32)
    limes16[:cfg.e] = np.asarray(LiMEs, dtype=np.float32)
    limes16[cfg.e] = np.asarray(LiME_shared, dtype=np.float32)
    gm = np.asarray(gamma, dtype=np.float32).reshape(1, 1)
    maps = []
    for c in range(cfg.n_cores):
        maps.append({
            "ids": ids_flat[c * cfg.tpc:(c + 1) * cfg.tpc].reshape(cfg.tpc, 1),
            "emb": emb_np,
            "a_t": a_t,
            "b_t": b_t,
            "limes": limes16,
            "gamma": gm,
        })
    return maps


def run(cfg, in_maps, **kwargs):
    nc = _get_program(cfg)
    return bass_utils.run_bass_kernel_spmd(
        nc, in_maps, core_ids=list(range(cfg.n_cores)), **kwargs)


def kernel(input_ids, emb_weight, A, B_lora, LiMEs, LiME_shared, gamma,
           **kwargs):
    cfg = FULL
    B, T = np.asarray(input_ids).shape
    scaling = 16.0 / 16.0  # ALPHA / RANK
    in_maps = make_in_maps(cfg, input_ids, emb_weight, A, B_lora, LiMEs,
                           LiME_shared, gamma, scaling)
    res = run(cfg, in_maps)
    out = np.concatenate([res.results[c]["out"] for c in range(cfg.n_cores)],
                         axis=0)
    return out.reshape(B, T, np.asarray(emb_weight).shape[1])



# revision 10
# speedup vs baseline: 1.3804x; 1.3804x over previous
"""Trainium2 Bass kernel: LoRA-LiME embedding with MoE routing.

Computes, for input_ids [B,T] over an embedding table [V,H]:
    E   = emb[ids]                                  # gather
    t   = E @ A.T ; delta = t @ B.T * scaling       # LoRA (rank 16)
    routing on first-8 feature slices with GLOBAL abs-max scales
    soft top-2 mask, renormalized expert weights
    p   = (1-g) * w @ LiMEs + g * LiME_shared       # g = sigmoid(gamma)
    out = E + delta * p
    (LiME_shared is folded in as expert row 8 with weight g.)

Sharding: data-parallel over the 8192 tokens (1024/core on 8 cores),
embedding table replicated. The global abs-max pair is an in-NEFF
AllReduce(max) over the 8 cores.

v2: all large matmuls run as float32r with moving free dim >= 256
(1 cycle/row vs 4 for fp32), stage-1 batched over 2-tile token groups,
startup DMAs consolidated, and the output is formed in PSUM:
pmix matmul -> in-place DVE multiply by stage-2 delta -> identity-
matmul accumulation of E -> single copy to SBUF -> DMA out.
"""

import ml_dtypes
import numpy as np

from concourse import bacc, bass, mybir, tile
from concourse import bass_utils
from concourse.masks import make_identity

F32 = mybir.dt.float32
BF16 = mybir.dt.bfloat16
I32 = mybir.dt.int32
ALU = mybir.AluOpType
ACTF = mybir.ActivationFunctionType
P = 128


class Cfg:
    def __init__(self, vocab=50257, h=2048, tpc=1024, n_cores=8, n_experts=8,
                 rank=16, gamma_routing=0.5, soft_topk_temp=0.5, eps=1e-6,
                 temperature=1.0):
        assert h % P == 0 and tpc % P == 0
        self.vocab, self.h, self.tpc, self.n_cores = vocab, h, tpc, n_cores
        self.e, self.r = n_experts, rank
        self.gamma_routing = gamma_routing
        self.soft_topk_temp = soft_topk_temp
        self.eps, self.temperature = eps, temperature
        self.nt = tpc // P                  # token tiles per core
        self.nch = h // P                   # 128-wide H chunks (transpose/stage1)
        self.ndc = (h + 511) // 512         # 512-wide N chunks (stage2/pmix)
        self.ng = self.nt // 2              # 2-tile stage-1 groups


FULL = Cfg()


def build_program(cfg: Cfg):
    nc = bacc.Bacc("TRN2", target_bir_lowering=False, debug=False,
                   num_devices=cfg.n_cores)

    ids = nc.dram_tensor("ids", [P, cfg.nt], I32, kind="ExternalInput").ap()
    emb = nc.dram_tensor("emb", [cfg.vocab, cfg.h], F32, kind="ExternalInput").ap()
    a_t = nc.dram_tensor("a_t", [P, cfg.nch * cfg.r], BF16,
                         kind="ExternalInput").ap()
    b_t = nc.dram_tensor("b_t", [cfg.r, cfg.h], BF16, kind="ExternalInput").ap()
    limes = nc.dram_tensor("limes", [16, cfg.h], BF16, kind="ExternalInput").ap()
    gamma = nc.dram_tensor("gamma", [1, 1], F32, kind="ExternalInput").ap()
    out = nc.dram_tensor("out", [cfg.tpc, cfg.h], F32, kind="ExternalOutput").ap()

    with tile.TileContext(nc) as tc:
        _body(nc, tc, cfg, ids, emb, a_t, b_t, limes, gamma, out)

    nc.compile()
    return nc


def _body(nc, tc, cfg, ids, emb, a_t, b_t, limes, gamma, out):
    E, R, H, NT, NCH, NDC, NG = (cfg.e, cfg.r, cfg.h, cfg.nt, cfg.nch, cfg.ndc,
                                 cfg.ng)
    inv_temp = 1.0 / cfg.temperature
    g_r = cfg.gamma_routing
    topk_slope = 1.0 / cfg.soft_topk_temp

    with (
        tc.tile_pool(name="const", bufs=1) as constp,
        tc.tile_pool(name="eout", bufs=NT) as eoutp,
        tc.tile_pool(name="eoT", bufs=2) as eoTp,
        tc.tile_pool(name="tT", bufs=NG) as tTp,
        tc.tile_pool(name="osb", bufs=4) as osbp,
        tc.tile_pool(name="small", bufs=4) as smallp,
        tc.tile_pool(name="ps_big", bufs=4, space="PSUM") as ps_big,
        tc.tile_pool(name="ps_tps", bufs=1, space="PSUM") as ps_tps,
        tc.tile_pool(name="ps_dsl", bufs=1, space="PSUM") as ps_dslp,
        tc.tile_pool(name="ps_sml", bufs=2, space="PSUM") as ps_sml,
        tc.tile_pool(name="dram", bufs=1, space="DRAM") as dramp,
    ):
        # ---- constants / params --------------------------------------
        ids_sb = constp.tile([P, NT], I32)
        nc.sync.dma_start(out=ids_sb[:], in_=ids[:, :])

        ident = constp.tile([P, P], F32)
        make_identity(nc, ident[:])
        ones1 = constp.tile([1, P], F32)
        nc.vector.memset(ones1[:], 1.0)

        # A^T chunks prearranged host-side: a_t[p, c*R+r] = A[r, c*128+p]
        aT_sb = constp.tile([P, NCH * R], BF16)
        nc.sync.dma_start(out=aT_sb[:], in_=a_t[:, :])
        bT_sb = constp.tile([16, H], BF16)
        nc.sync.dma_start(out=bT_sb[:R, :], in_=b_t[:, :])
        limes_sb = constp.tile([16, H], BF16)
        nc.sync.dma_start(out=limes_sb[:], in_=limes[:, :])
        gm1 = constp.tile([1, 1], F32)
        nc.sync.dma_start(out=gm1[:], in_=gamma[:, :])

        # g = sigmoid(gamma) broadcast to all 128 partitions via K=1 matmul
        ps_g = ps_sml.tile([P, 2], F32, tag="ps_sml")
        nc.tensor.matmul(out=ps_g[:, 0:1], lhsT=ones1[:], rhs=gm1[:],
                         start=True, stop=True)
        gb = smallp.tile([P, 1], F32, tag="gtiles")
        nc.scalar.activation(out=gb[:], in_=ps_g[:, 0:1], func=ACTF.Sigmoid)
        omg = smallp.tile([P, 1], F32, tag="gtiles")
        nc.vector.tensor_scalar(out=omg[:], in0=gb[:], scalar1=-1.0, scalar2=1.0,
                                op0=ALU.mult, op1=ALU.add)

        # ---- resident accumulators -----------------------------------
        esl_all = constp.tile([P, NT * E], F32)      # E[:, :8] per tile
        dsl_ps = ps_dslp.tile([P, NT * E], F32, tag="ps_dsl", bufs=1)
        eouts, tTs = [], []

        # ================= phase A: gather + transpose + stage1 =======
        for g in range(NG):
            # eoT group layout: [p, ch(16), j(2), tok(128)] so stage-1's
            # rhs for chunk ch is the contiguous 256-wide slice at ch*256.
            eoT = eoTp.tile([P, NCH * 2 * P], BF16, tag="eoT", name=f"eoT{g}")
            for j in range(2):
                i = g * 2 + j
                eo = eoutp.tile([P, H], F32, tag="eout", name=f"eout{i}")
                eouts.append(eo)
                nc.gpsimd.indirect_dma_start(
                    out=eo[:], out_offset=None, in_=emb,
                    in_offset=bass.IndirectOffsetOnAxis(ap=ids_sb[:, i:i + 1],
                                                        axis=0))

                nc.vector.tensor_copy(out=esl_all[:, i * E:(i + 1) * E],
                                      in_=eo[:, 0:E])

                # transpose H chunks, 4 per PSUM buffer
                eoT_v = eoT[:].rearrange("p (c j k) -> p c j k", c=NCH, j=2)
                for c4 in range(NCH // 4):
                    ps_tr = ps_big.tile([P, 4 * P], F32, tag="ps_big",
                                        name=f"ps_tr{i}_{c4}")
                    for q in range(4):
                        ch = c4 * 4 + q
                        nc.tensor.transpose(
                            out=ps_tr[:, q * P:(q + 1) * P],
                            in_=eo[:, ch * P:(ch + 1) * P],
                            identity=ident[:])
                    dst = eoT_v[:, c4 * 4:(c4 + 1) * 4, j, :]
                    if c4 % 2 == 0:
                        nc.scalar.copy(out=dst, in_=ps_tr[:])
                    else:
                        nc.vector.tensor_copy(out=dst, in_=ps_tr[:])

            # stage 1 over the 2-tile group: t^T [16, 256]
            tps = ps_tps.tile([16, 2 * P], F32, tag="tps", name=f"tps{g}")
            for ch in range(NCH):
                nc.tensor.matmul(
                    out=tps[:R, :],
                    lhsT=aT_sb[:, ch * R:(ch + 1) * R],
                    rhs=eoT[:, ch * 2 * P:(ch + 1) * 2 * P],
                    start=(ch == 0), stop=(ch == NCH - 1))
            tT = tTp.tile([16, 2 * P], BF16, tag="tT", name=f"tT{g}")
            tTs.append(tT)
            nc.vector.tensor_copy(out=tT[:], in_=tps[:])

            # d_sl for both tiles (needs only b_t cols 0:E)
            for j in range(2):
                i = g * 2 + j
                nc.tensor.matmul(out=dsl_ps[:, i * E:(i + 1) * E],
                                 lhsT=tT[:R, j * P:(j + 1) * P],
                                 rhs=bT_sb[:R, 0:E],
                                 start=True, stop=True)

        # ================= phase B: global abs-max ====================
        dsl_sb = constp.tile([P, NT * E], F32)
        nc.vector.tensor_copy(out=dsl_sb[:], in_=dsl_ps[:])

        loc2 = smallp.tile([P, 2], F32, tag="loc")
        nc.vector.tensor_reduce(out=loc2[:, 0:1], in_=esl_all[:],
                                axis=mybir.AxisListType.X, op=ALU.max,
                                apply_absolute_value=True)
        nc.vector.tensor_reduce(out=loc2[:, 1:2], in_=dsl_sb[:],
                                axis=mybir.AxisListType.X, op=ALU.max,
                                apply_absolute_value=True)
        ps_l = ps_sml.tile([2, P], F32, tag="ps_sml")
        nc.tensor.transpose(out=ps_l[:], in_=loc2[:], identity=ident[:])
        l2T = smallp.tile([2, P], F32, tag="loc")
        nc.vector.tensor_copy(out=l2T[:], in_=ps_l[:])
        lmax = smallp.tile([2, 1], F32, tag="loc")
        nc.vector.tensor_reduce(out=lmax[:], in_=l2T[:],
                                axis=mybir.AxisListType.X, op=ALU.max)

        deltas = []

        cc_in = dramp.tile([2, 1], F32)
        cc_out = dramp.tile(
            [2, 1], F32,
            addr_space="Shared" if cfg.n_cores > 4 else "Local")
        nc.sync.dma_start(out=cc_in[:], in_=lmax[:])
        nc.gpsimd.collective_compute(
            "AllReduce", ALU.max,
            replica_groups=[list(range(cfg.n_cores))],
            ins=[cc_in[:].opt()], outs=[cc_out[:].opt()])
        sc01 = smallp.tile([1, 2], F32, tag="loc")
        nc.sync.dma_start(out=sc01[:], in_=cc_out[:].rearrange("a b -> b a"))

        # broadcast scales to 128 partitions; sc2 = g_r/temp/max(scale, eps)
        ps_b = ps_sml.tile([P, 2], F32, tag="ps_sml")
        nc.tensor.matmul(out=ps_b[:], lhsT=ones1[:], rhs=sc01[:],
                         start=True, stop=True)
        sc2 = smallp.tile([P, 2], F32, tag="loc")
        nc.vector.tensor_scalar_max(sc2[:], ps_b[:], cfg.eps)
        nc.vector.reciprocal(out=sc2[:], in_=sc2[:])
        nc.vector.tensor_scalar_mul(sc2[:], sc2[:], g_r * inv_temp)
        sc2e = smallp.tile([P, 2], F32, tag="loc")
        nc.vector.tensor_scalar_mul(sc2e[:], sc2[:], (1.0 - g_r) / g_r)

        # ============ phase B2: stage-2 delta (overlaps collective) ===
        for i in range(NT):
            tT = tTs[i // 2]
            j = i % 2
            dl = eoutp.tile([P, H], F32, tag="delta", bufs=NT,
                            name=f"delta{i}")
            deltas.append(dl)
            for c in range(NDC):
                n0, n1 = c * 512, min((c + 1) * 512, H)
                ps_d2 = ps_big.tile([P, n1 - n0], F32, tag="ps_big",
                                    name=f"ps_d2{i}_{c}")
                nc.tensor.matmul(out=ps_d2[:],
                                 lhsT=tT[:R, j * P:(j + 1) * P],
                                 rhs=bT_sb[:R, n0:n1],
                                 start=True, stop=True)
                if c % 4 == 3:
                    nc.vector.tensor_copy(out=dl[:, n0:n1], in_=ps_d2[:])
                else:
                    nc.scalar.copy(out=dl[:, n0:n1], in_=ps_d2[:])

        # ================= phase C: routing + output ==================
        dsc = constp.tile([P, NT * E], F32)
        nc.vector.tensor_scalar(out=dsc[:], in0=dsl_sb[:], scalar1=sc2[:, 1:2],
                                scalar2=None, op0=ALU.mult)
        logits = constp.tile([P, NT * E], F32)
        nc.vector.scalar_tensor_tensor(out=logits[:], in0=esl_all[:],
                                       scalar=sc2e[:, 0:1], in1=dsc[:],
                                       op0=ALU.mult, op1=ALU.add)
        e_all = constp.tile([P, NT * E], F32)
        nc.scalar.activation(out=e_all[:], in_=logits[:], func=ACTF.Exp)
        s8 = smallp.tile([P, NT], F32, tag="r8")
        nc.vector.tensor_reduce(
            out=s8[:], in_=e_all[:].rearrange("p (t e) -> p t e", e=E),
            axis=mybir.AxisListType.X, op=ALU.add)
        rs2 = smallp.tile([P, NT], F32, tag="r8")
        nc.vector.reciprocal(out=rs2[:], in_=s8[:])
        nc.vector.tensor_scalar_mul(rs2[:], rs2[:], topk_slope)

        thr = smallp.tile([P, NT], F32, tag="r8")
        for i in range(NT):
            m8 = smallp.tile([P, 8], F32, tag="m8", bufs=2, name=f"m8_{i}")
            nc.vector.max(out=m8[:], in_=e_all[:, i * E:(i + 1) * E])
            nc.vector.tensor_copy(out=thr[:, i:i + 1], in_=m8[:, 1:2])
        bias_all = smallp.tile([P, NT], F32, tag="r8")
        nc.vector.tensor_mul(bias_all[:], thr[:], rs2[:])
        nc.vector.tensor_scalar_mul(bias_all[:], bias_all[:], -1.0)

        mask_all = constp.tile([P, NT * E], F32)
        for i in range(NT):
            nc.scalar.activation(out=mask_all[:, i * E:(i + 1) * E],
                                 in_=e_all[:, i * E:(i + 1) * E],
                                 func=ACTF.Sigmoid,
                                 bias=bias_all[:, i:i + 1],
                                 scale=rs2[:, i:i + 1])
        u_all = constp.tile([P, NT * E], F32)
        nc.vector.scalar_tensor_tensor(out=u_all[:], in0=e_all[:], scalar=0.0,
                                       in1=mask_all[:], op0=ALU.bypass,
                                       op1=ALU.mult)
        su8 = smallp.tile([P, NT], F32, tag="r8")
        nc.vector.tensor_reduce(
            out=su8[:], in_=u_all[:].rearrange("p (t e) -> p t e", e=E),
            axis=mybir.AxisListType.X, op=ALU.add)
        den8 = smallp.tile([P, NT], F32, tag="r8")
        nc.vector.scalar_tensor_tensor(out=den8[:], in0=s8[:], scalar=1e-9,
                                       in1=su8[:], op0=ALU.mult, op1=ALU.add)
        rdg8 = smallp.tile([P, NT], F32, tag="r8")
        nc.vector.reciprocal(out=rdg8[:], in_=den8[:])
        nc.vector.tensor_mul(rdg8[:], rdg8[:],
                             omg[:].to_broadcast([P, NT]))

        for i in range(NT):
            w16 = smallp.tile([P, 16], F32, tag="w16", name=f"w16_{i}")
            nc.vector.memset(w16[:], 0.0)
            nc.vector.tensor_scalar(out=w16[:, 0:E], in0=u_all[:, i * E:(i + 1) * E],
                                    scalar1=rdg8[:, i:i + 1], scalar2=None,
                                    op0=ALU.mult)
            nc.vector.tensor_copy(out=w16[:, E:E + 1], in_=gb[:])
            ps_w = ps_sml.tile([16, P], F32, tag="ps_sml", name=f"ps_w{i}")
            nc.tensor.transpose(out=ps_w[:], in_=w16[:], identity=ident[:])
            wT = smallp.tile([16, P], BF16, tag="wT", bufs=2, name=f"wT{i}")
            nc.scalar.copy(out=wT[:], in_=ps_w[:])

            eo, dl = eouts[i], deltas[i]
            for c in range(NDC):
                n0, n1 = c * 512, min((c + 1) * 512, H)
                # pmix for this chunk; multiply reads the PSUM directly
                ps_pm = ps_big.tile([P, n1 - n0], F32, tag="ps_big",
                                    name=f"ps_pm{i}_{c}")
                nc.tensor.matmul(out=ps_pm[:], lhsT=wT[:],
                                 rhs=limes_sb[:, n0:n1],
                                 start=True, stop=True)
                osb = osbp.tile([P, n1 - n0], F32, tag="osb",
                                name=f"osb{i}_{c}")
                nc.vector.tensor_mul(osb[:], ps_pm[:], dl[:, n0:n1])
                if c % 2 == 0:
                    nc.gpsimd.tensor_add(osb[:], osb[:], eo[:, n0:n1])
                else:
                    nc.vector.tensor_add(osb[:], osb[:], eo[:, n0:n1])
                nc.sync.dma_start(out=out[i * P:(i + 1) * P, n0:n1],
                                  in_=osb[:])


# ---------------------------------------------------------------------
# host entry point
# ---------------------------------------------------------------------
_CACHED = {}


def _get_program(cfg: Cfg):
    key = (cfg.vocab, cfg.h, cfg.tpc, cfg.n_cores)
    if key not in _CACHED:
        _CACHED[key] = build_program(cfg)
    return _CACHED[key]


def make_in_maps(cfg, input_ids, emb_weight, A, B_lora, LiMEs, LiME_shared, gamma,
                 scaling):
    ids_flat = np.asarray(input_ids).reshape(-1).astype(np.int32)
    emb_np = np.ascontiguousarray(np.asarray(emb_weight, dtype=np.float32))
    a_np = np.asarray(A, dtype=np.float32)           # [R, H]
    # a_t[p, c*R + r] = A[r, c*128 + p]
    a_t = np.ascontiguousarray(
        a_np.T.reshape(cfg.nch, P, cfg.r).transpose(1, 0, 2).reshape(
            P, cfg.nch * cfg.r)).astype(ml_dtypes.bfloat16)
    b_t = np.ascontiguousarray(
        np.asarray(B_lora, dtype=np.float32).T * scaling).astype(
            ml_dtypes.bfloat16)
    limes16 = np.zeros((16, cfg.h), dtype=np.float32)
    limes16[:cfg.e] = np.asarray(LiMEs, dtype=np.float32)
    limes16[cfg.e] = np.asarray(LiME_shared, dtype=np.float32)
    limes16 = limes16.astype(ml_dtypes.bfloat16)
    gm = np.asarray(gamma, dtype=np.float32).reshape(1, 1)
    maps = []
    for c in range(cfg.n_cores):
        ids_core = ids_flat[c * cfg.tpc:(c + 1) * cfg.tpc]
        ids_pm = np.ascontiguousarray(ids_core.reshape(cfg.nt, P).T)
        maps.append({
            "ids": ids_pm,
            "emb": emb_np,
            "a_t": a_t,
            "b_t": b_t,
            "limes": limes16,
            "gamma": gm,
        })
    return maps


def run(cfg, in_maps, **kwargs):
    nc = _get_program(cfg)
    return bass_utils.run_bass_kernel_spmd(
        nc, in_maps, core_ids=list(range(cfg.n_cores)), **kwargs)


def kernel(input_ids, emb_weight, A, B_lora, LiMEs, LiME_shared, gamma,
           **kwargs):
    cfg = FULL
    B, T = np.asarray(input_ids).shape
    scaling = 16.0 / 16.0  # ALPHA / RANK
    in_maps = make_in_maps(cfg, input_ids, emb_weight, A, B_lora, LiMEs,
                           LiME_shared, gamma, scaling)
    res = run(cfg, in_maps)
    out = np.concatenate([res.results[c]["out"] for c in range(cfg.n_cores)],
                         axis=0)
    return out.reshape(B, T, np.asarray(emb_weight).shape[1])


# revision 11
# speedup vs baseline: 1.5638x; 1.1329x over previous
"""Trainium2 Bass kernel: LoRA-LiME embedding with MoE routing.

Computes, for input_ids [B,T] over an embedding table [V,H]:
    E   = emb[ids]                                  # gather
    t   = E @ A.T ; delta = t @ B.T * scaling       # LoRA (rank 16)
    routing on first-8 feature slices with GLOBAL abs-max scales
    soft top-2 mask, renormalized expert weights
    p   = (1-g) * w @ LiMEs + g * LiME_shared       # g = sigmoid(gamma)
    out = E + delta * p
    (LiME_shared is folded in as expert row 8 with weight g.)

Sharding: data-parallel over the 8192 tokens (1024/core on 8 cores),
embedding table replicated. The global abs-max pair is an in-NEFF
AllReduce(max) over the 8 cores.

v2: all large matmuls run as float32r with moving free dim >= 256
(1 cycle/row vs 4 for fp32), stage-1 batched over 2-tile token groups,
startup DMAs consolidated, and the output is formed in PSUM:
pmix matmul -> in-place DVE multiply by stage-2 delta -> identity-
matmul accumulation of E -> single copy to SBUF -> DMA out.
"""

import ml_dtypes
import numpy as np

from concourse import bacc, bass, mybir, tile
from concourse import bass_utils
from concourse.masks import make_identity

F32 = mybir.dt.float32
BF16 = mybir.dt.bfloat16
I32 = mybir.dt.int32
ALU = mybir.AluOpType
ACTF = mybir.ActivationFunctionType
P = 128


class Cfg:
    def __init__(self, vocab=50257, h=2048, tpc=1024, n_cores=8, n_experts=8,
                 rank=16, gamma_routing=0.5, soft_topk_temp=0.5, eps=1e-6,
                 temperature=1.0):
        assert h % P == 0 and tpc % P == 0
        self.vocab, self.h, self.tpc, self.n_cores = vocab, h, tpc, n_cores
        self.e, self.r = n_experts, rank
        self.gamma_routing = gamma_routing
        self.soft_topk_temp = soft_topk_temp
        self.eps, self.temperature = eps, temperature
        self.nt = tpc // P                  # token tiles per core
        self.nch = h // P                   # 128-wide H chunks (transpose/stage1)
        self.ndc = (h + 511) // 512         # 512-wide N chunks (stage2/pmix)
        self.ng = self.nt // 2              # 2-tile stage-1 groups


FULL = Cfg()


def build_program(cfg: Cfg):
    nc = bacc.Bacc("TRN2", target_bir_lowering=False, debug=False,
                   num_devices=cfg.n_cores)

    ids = nc.dram_tensor("ids", [P, cfg.nt], I32, kind="ExternalInput").ap()
    emb = nc.dram_tensor("emb", [cfg.vocab, cfg.h], F32, kind="ExternalInput").ap()
    a_t = nc.dram_tensor("a_t", [P, cfg.nch * cfg.r], BF16,
                         kind="ExternalInput").ap()
    b_t = nc.dram_tensor("b_t", [cfg.r, cfg.h], BF16, kind="ExternalInput").ap()
    limes = nc.dram_tensor("limes", [16, cfg.h], BF16, kind="ExternalInput").ap()
    gamma = nc.dram_tensor("gamma", [1, 1], F32, kind="ExternalInput").ap()
    out = nc.dram_tensor("out", [cfg.tpc, cfg.h], F32, kind="ExternalOutput").ap()

    with tile.TileContext(nc) as tc:
        _body(nc, tc, cfg, ids, emb, a_t, b_t, limes, gamma, out)

    nc.compile()
    return nc


def _body(nc, tc, cfg, ids, emb, a_t, b_t, limes, gamma, out):
    E, R, H, NT, NCH, NDC, NG = (cfg.e, cfg.r, cfg.h, cfg.nt, cfg.nch, cfg.ndc,
                                 cfg.ng)
    inv_temp = 1.0 / cfg.temperature
    g_r = cfg.gamma_routing
    topk_slope = 1.0 / cfg.soft_topk_temp

    with (
        tc.tile_pool(name="const", bufs=1) as constp,
        tc.tile_pool(name="eout", bufs=NT) as eoutp,
        tc.tile_pool(name="eoT", bufs=2) as eoTp,
        tc.tile_pool(name="tT", bufs=NG) as tTp,
        tc.tile_pool(name="osb", bufs=4) as osbp,
        tc.tile_pool(name="small", bufs=4) as smallp,
        tc.tile_pool(name="ps_big", bufs=4, space="PSUM") as ps_big,
        tc.tile_pool(name="ps_tps", bufs=1, space="PSUM") as ps_tps,
        tc.tile_pool(name="ps_dsl", bufs=1, space="PSUM") as ps_dslp,
        tc.tile_pool(name="ps_sml", bufs=2, space="PSUM") as ps_sml,
        tc.tile_pool(name="dram", bufs=1, space="DRAM") as dramp,
    ):
        # ---- constants / params --------------------------------------
        ids_sb = constp.tile([P, NT], I32)
        nc.sync.dma_start(out=ids_sb[:], in_=ids[:, :])

        ident = constp.tile([P, P], F32)
        make_identity(nc, ident[:])
        ident_bf = constp.tile([P, P], BF16)
        make_identity(nc, ident_bf[:])
        ones1 = constp.tile([1, P], F32)
        nc.vector.memset(ones1[:], 1.0)

        # A^T chunks prearranged host-side: a_t[p, c*R+r] = A[r, c*128+p]
        aT_sb = constp.tile([P, NCH * R], BF16)
        nc.sync.dma_start(out=aT_sb[:], in_=a_t[:, :])
        bT_sb = constp.tile([16, H], BF16)
        nc.sync.dma_start(out=bT_sb[:R, :], in_=b_t[:, :])
        limes_sb = constp.tile([16, H], BF16)
        nc.sync.dma_start(out=limes_sb[:], in_=limes[:, :])
        gm1 = constp.tile([1, 1], F32)
        nc.sync.dma_start(out=gm1[:], in_=gamma[:, :])

        # g = sigmoid(gamma) broadcast to all 128 partitions via K=1 matmul
        ps_g = ps_sml.tile([P, 2], F32, tag="ps_sml")
        nc.tensor.matmul(out=ps_g[:, 0:1], lhsT=ones1[:], rhs=gm1[:],
                         start=True, stop=True)
        gb = smallp.tile([P, 1], F32, tag="gtiles")
        nc.scalar.activation(out=gb[:], in_=ps_g[:, 0:1], func=ACTF.Sigmoid)
        omg = smallp.tile([P, 1], F32, tag="gtiles")
        nc.vector.tensor_scalar(out=omg[:], in0=gb[:], scalar1=-1.0, scalar2=1.0,
                                op0=ALU.mult, op1=ALU.add)

        # ---- resident accumulators -----------------------------------
        esl_all = constp.tile([P, NT * E], F32)      # E[:, :8] per tile
        dsl_ps = ps_dslp.tile([P, NT * E], F32, tag="ps_dsl", bufs=1)
        eouts, tTs = [], []

        # ================= phase A: gather + transpose + stage1 =======
        for g in range(NG):
            # eoT group layout: [p, ch(16), j(2), tok(128)] so stage-1's
            # rhs for chunk ch is the contiguous 256-wide slice at ch*256.
            eoT = eoTp.tile([P, NCH * 2 * P], BF16, tag="eoT", name=f"eoT{g}")
            for j in range(2):
                i = g * 2 + j
                eo = eoutp.tile([P, H], F32, tag="eout", name=f"eout{i}")
                eouts.append(eo)
                nc.gpsimd.indirect_dma_start(
                    out=eo[:], out_offset=None, in_=emb,
                    in_offset=bass.IndirectOffsetOnAxis(ap=ids_sb[:, i:i + 1],
                                                        axis=0))

                nc.vector.tensor_copy(out=esl_all[:, i * E:(i + 1) * E],
                                      in_=eo[:, 0:E])

                # bf16 copy of E for the transposes (1 cycle/row on PE);
                # halves are converted on scalar and vector in parallel
                eo_bf = eoTp.tile([P, H], BF16, tag="eo_bf", bufs=3,
                                  name=f"eo_bf{i}")
                nc.scalar.copy(out=eo_bf[:, 0:H // 2], in_=eo[:, 0:H // 2])
                nc.vector.tensor_copy(out=eo_bf[:, H // 2:], in_=eo[:, H // 2:])

                # transpose H chunks, 4 per PSUM buffer
                eoT_v = eoT[:].rearrange("p (c j k) -> p c j k", c=NCH, j=2)
                for c4 in range(NCH // 4):
                    ps_tr = ps_big.tile([P, 4 * P], BF16, tag="ps_big",
                                        name=f"ps_tr{i}_{c4}")
                    for q in range(4):
                        ch = c4 * 4 + q
                        nc.tensor.transpose(
                            out=ps_tr[:, q * P:(q + 1) * P],
                            in_=eo_bf[:, ch * P:(ch + 1) * P],
                            identity=ident_bf[:])
                    dst = eoT_v[:, c4 * 4:(c4 + 1) * 4, j, :]
                    if c4 % 2 == 0:
                        nc.scalar.copy(out=dst, in_=ps_tr[:])
                    else:
                        nc.vector.tensor_copy(out=dst, in_=ps_tr[:])

            # stage 1 over the 2-tile group: t^T [16, 256]
            tps = ps_tps.tile([16, 2 * P], F32, tag="tps", name=f"tps{g}")
            for ch in range(NCH):
                nc.tensor.matmul(
                    out=tps[:R, :],
                    lhsT=aT_sb[:, ch * R:(ch + 1) * R],
                    rhs=eoT[:, ch * 2 * P:(ch + 1) * 2 * P],
                    start=(ch == 0), stop=(ch == NCH - 1))
            tT = tTp.tile([16, 2 * P], BF16, tag="tT", name=f"tT{g}")
            tTs.append(tT)
            nc.vector.tensor_copy(out=tT[:], in_=tps[:])

            # d_sl for both tiles (needs only b_t cols 0:E)
            for j in range(2):
                i = g * 2 + j
                nc.tensor.matmul(out=dsl_ps[:, i * E:(i + 1) * E],
                                 lhsT=tT[:R, j * P:(j + 1) * P],
                                 rhs=bT_sb[:R, 0:E],
                                 start=True, stop=True)

        # ================= phase B: global abs-max ====================
        dsl_sb = constp.tile([P, NT * E], F32)
        nc.vector.tensor_copy(out=dsl_sb[:], in_=dsl_ps[:])

        loc2 = smallp.tile([P, 2], F32, tag="loc")
        nc.vector.tensor_reduce(out=loc2[:, 0:1], in_=esl_all[:],
                                axis=mybir.AxisListType.X, op=ALU.max,
                                apply_absolute_value=True)
        nc.vector.tensor_reduce(out=loc2[:, 1:2], in_=dsl_sb[:],
                                axis=mybir.AxisListType.X, op=ALU.max,
                                apply_absolute_value=True)
        ps_l = ps_sml.tile([2, P], F32, tag="ps_sml")
        nc.tensor.transpose(out=ps_l[:], in_=loc2[:], identity=ident[:])
        l2T = smallp.tile([2, P], F32, tag="loc")
        nc.vector.tensor_copy(out=l2T[:], in_=ps_l[:])
        lmax = smallp.tile([2, 1], F32, tag="loc")
        nc.vector.tensor_reduce(out=lmax[:], in_=l2T[:],
                                axis=mybir.AxisListType.X, op=ALU.max)

        deltas = []

        cc_in = dramp.tile([2, 1], F32)
        cc_out = dramp.tile(
            [2, 1], F32,
            addr_space="Shared" if cfg.n_cores > 4 else "Local")
        nc.sync.dma_start(out=cc_in[:], in_=lmax[:])
        nc.gpsimd.collective_compute(
            "AllReduce", ALU.max,
            replica_groups=[list(range(cfg.n_cores))],
            ins=[cc_in[:].opt()], outs=[cc_out[:].opt()])
        sc01 = smallp.tile([1, 2], F32, tag="loc")
        nc.sync.dma_start(out=sc01[:], in_=cc_out[:].rearrange("a b -> b a"))

        # broadcast scales to 128 partitions; sc2 = g_r/temp/max(scale, eps)
        ps_b = ps_sml.tile([P, 2], F32, tag="ps_sml")
        nc.tensor.matmul(out=ps_b[:], lhsT=ones1[:], rhs=sc01[:],
                         start=True, stop=True)
        sc2 = smallp.tile([P, 2], F32, tag="loc")
        nc.vector.tensor_scalar_max(sc2[:], ps_b[:], cfg.eps)
        nc.vector.reciprocal(out=sc2[:], in_=sc2[:])
        nc.vector.tensor_scalar_mul(sc2[:], sc2[:], g_r * inv_temp)
        sc2e = smallp.tile([P, 2], F32, tag="loc")
        nc.vector.tensor_scalar_mul(sc2e[:], sc2[:], (1.0 - g_r) / g_r)

        # ============ phase B2: stage-2 delta (overlaps collective) ===
        for i in range(NT):
            tT = tTs[i // 2]
            j = i % 2
            dl = eoutp.tile([P, H], BF16, tag="delta", bufs=NT,
                            name=f"delta{i}")
            deltas.append(dl)
            for c in range(NDC):
                n0, n1 = c * 512, min((c + 1) * 512, H)
                ps_d2 = ps_big.tile([P, n1 - n0], F32, tag="ps_big",
                                    name=f"ps_d2{i}_{c}")
                nc.tensor.matmul(out=ps_d2[:],
                                 lhsT=tT[:R, j * P:(j + 1) * P],
                                 rhs=bT_sb[:R, n0:n1],
                                 start=True, stop=True)
                if c % 4 == 3:
                    nc.vector.tensor_copy(out=dl[:, n0:n1], in_=ps_d2[:])
                else:
                    nc.scalar.copy(out=dl[:, n0:n1], in_=ps_d2[:])

        # ================= phase C: routing + output ==================
        dsc = constp.tile([P, NT * E], F32)
        nc.vector.tensor_scalar(out=dsc[:], in0=dsl_sb[:], scalar1=sc2[:, 1:2],
                                scalar2=None, op0=ALU.mult)
        logits = constp.tile([P, NT * E], F32)
        nc.vector.scalar_tensor_tensor(out=logits[:], in0=esl_all[:],
                                       scalar=sc2e[:, 0:1], in1=dsc[:],
                                       op0=ALU.mult, op1=ALU.add)
        e_all = constp.tile([P, NT * E], F32)
        nc.scalar.activation(out=e_all[:], in_=logits[:], func=ACTF.Exp)
        s8 = smallp.tile([P, NT], F32, tag="r8")
        nc.vector.tensor_reduce(
            out=s8[:], in_=e_all[:].rearrange("p (t e) -> p t e", e=E),
            axis=mybir.AxisListType.X, op=ALU.add)
        rs2 = smallp.tile([P, NT], F32, tag="r8")
        nc.vector.reciprocal(out=rs2[:], in_=s8[:])
        nc.vector.tensor_scalar_mul(rs2[:], rs2[:], topk_slope)

        thr = smallp.tile([P, NT], F32, tag="r8")
        for i in range(NT):
            m8 = smallp.tile([P, 8], F32, tag="m8", bufs=2, name=f"m8_{i}")
            nc.vector.max(out=m8[:], in_=e_all[:, i * E:(i + 1) * E])
            nc.vector.tensor_copy(out=thr[:, i:i + 1], in_=m8[:, 1:2])
        bias_all = smallp.tile([P, NT], F32, tag="r8")
        nc.vector.tensor_mul(bias_all[:], thr[:], rs2[:])
        nc.vector.tensor_scalar_mul(bias_all[:], bias_all[:], -1.0)

        mask_all = constp.tile([P, NT * E], F32)
        for i in range(NT):
            nc.scalar.activation(out=mask_all[:, i * E:(i + 1) * E],
                                 in_=e_all[:, i * E:(i + 1) * E],
                                 func=ACTF.Sigmoid,
                                 bias=bias_all[:, i:i + 1],
                                 scale=rs2[:, i:i + 1])
        u_all = constp.tile([P, NT * E], F32)
        nc.vector.scalar_tensor_tensor(out=u_all[:], in0=e_all[:], scalar=0.0,
                                       in1=mask_all[:], op0=ALU.bypass,
                                       op1=ALU.mult)
        su8 = smallp.tile([P, NT], F32, tag="r8")
        nc.vector.tensor_reduce(
            out=su8[:], in_=u_all[:].rearrange("p (t e) -> p t e", e=E),
            axis=mybir.AxisListType.X, op=ALU.add)
        den8 = smallp.tile([P, NT], F32, tag="r8")
        nc.vector.scalar_tensor_tensor(out=den8[:], in0=s8[:], scalar=1e-9,
                                       in1=su8[:], op0=ALU.mult, op1=ALU.add)
        rdg8 = smallp.tile([P, NT], F32, tag="r8")
        nc.vector.reciprocal(out=rdg8[:], in_=den8[:])
        nc.vector.tensor_mul(rdg8[:], rdg8[:],
                             omg[:].to_broadcast([P, NT]))

        for i in range(NT):
            w16 = smallp.tile([P, 16], F32, tag="w16", name=f"w16_{i}")
            nc.vector.memset(w16[:], 0.0)
            nc.vector.tensor_scalar(out=w16[:, 0:E], in0=u_all[:, i * E:(i + 1) * E],
                                    scalar1=rdg8[:, i:i + 1], scalar2=None,
                                    op0=ALU.mult)
            nc.vector.tensor_copy(out=w16[:, E:E + 1], in_=gb[:])
            ps_w = ps_sml.tile([16, P], F32, tag="ps_sml", name=f"ps_w{i}")
            nc.tensor.transpose(out=ps_w[:], in_=w16[:], identity=ident[:])
            wT = smallp.tile([16, P], BF16, tag="wT", bufs=2, name=f"wT{i}")
            nc.scalar.copy(out=wT[:], in_=ps_w[:])

            eo, dl = eouts[i], deltas[i]
            for c in range(NDC):
                n0, n1 = c * 512, min((c + 1) * 512, H)
                # pmix for this chunk; multiply reads the PSUM directly
                ps_pm = ps_big.tile([P, n1 - n0], F32, tag="ps_big",
                                    name=f"ps_pm{i}_{c}")
                nc.tensor.matmul(out=ps_pm[:], lhsT=wT[:],
                                 rhs=limes_sb[:, n0:n1],
                                 start=True, stop=True)
                pm_sb = osbp.tile([P, n1 - n0], BF16, tag="pm_sb",
                                  bufs=3, name=f"pm_sb{i}_{c}")
                nc.scalar.copy(out=pm_sb[:], in_=ps_pm[:])
                tmp_bf = osbp.tile([P, n1 - n0], BF16, tag="tmp_bf",
                                   bufs=3, name=f"tmp_bf{i}_{c}")
                nc.vector.tensor_mul(tmp_bf[:], dl[:, n0:n1], pm_sb[:])
                osb = osbp.tile([P, n1 - n0], F32, tag="osb",
                                name=f"osb{i}_{c}")
                if c % 4 == 1:
                    nc.gpsimd.tensor_add(osb[:], tmp_bf[:], eo[:, n0:n1])
                else:
                    nc.vector.tensor_add(osb[:], tmp_bf[:], eo[:, n0:n1])
                nc.sync.dma_start(out=out[i * P:(i + 1) * P, n0:n1],
                                  in_=osb[:])


# ---------------------------------------------------------------------
# host entry point
# ---------------------------------------------------------------------
_CACHED = {}


def _get_program(cfg: Cfg):
    key = (cfg.vocab, cfg.h, cfg.tpc, cfg.n_cores)
    if key not in _CACHED:
        _CACHED[key] = build_program(cfg)
    return _CACHED[key]


def make_in_maps(cfg, input_ids, emb_weight, A, B_lora, LiMEs, LiME_shared, gamma,
                 scaling):
    ids_flat = np.asarray(input_ids).reshape(-1).astype(np.int32)
    emb_np = np.ascontiguousarray(np.asarray(emb_weight, dtype=np.float32))
    a_np = np.asarray(A, dtype=np.float32)           # [R, H]
    # a_t[p, c*R + r] = A[r, c*128 + p]
    a_t = np.ascontiguousarray(
        a_np.T.reshape(cfg.nch, P, cfg.r).transpose(1, 0, 2).reshape(
            P, cfg.nch * cfg.r)).astype(ml_dtypes.bfloat16)
    b_t = np.ascontiguousarray(
        np.asarray(B_lora, dtype=np.float32).T * scaling).astype(
            ml_dtypes.bfloat16)
    limes16 = np.zeros((16, cfg.h), dtype=np.float32)
    limes16[:cfg.e] = np.asarray(LiMEs, dtype=np.float32)
    limes16[cfg.e] = np.asarray(LiME_shared, dtype=np.float32)
    limes16 = limes16.astype(ml_dtypes.bfloat16)
    gm = np.asarray(gamma, dtype=np.float32).reshape(1, 1)
    maps = []
    for c in range(cfg.n_cores):
        ids_core = ids_flat[c * cfg.tpc:(c + 1) * cfg.tpc]
        ids_pm = np.ascontiguousarray(ids_core.reshape(cfg.nt, P).T)
        maps.append({
            "ids": ids_pm,
            "emb": emb_np,
            "a_t": a_t,
            "b_t": b_t,
            "limes": limes16,
            "gamma": gm,
        })
    return maps


def run(cfg, in_maps, **kwargs):
    nc = _get_program(cfg)
    return bass_utils.run_bass_kernel_spmd(
        nc, in_maps, core_ids=list(range(cfg.n_cores)), **kwargs)


def kernel(input_ids, emb_weight, A, B_lora, LiMEs, LiME_shared, gamma,
           **kwargs):
    cfg = FULL
    B, T = np.asarray(input_ids).shape
    scaling = 16.0 / 16.0  # ALPHA / RANK
    in_maps = make_in_maps(cfg, input_ids, emb_weight, A, B_lora, LiMEs,
                           LiME_shared, gamma, scaling)
    res = run(cfg, in_maps)
    out = np.concatenate([res.results[c]["out"] for c in range(cfg.n_cores)],
                         axis=0)
    return out.reshape(B, T, np.asarray(emb_weight).shape[1])
